# revision 6
# baseline (speedup 1.0000x reference)
"""FFSNN forward scan on 8 Trainium2 NeuronCores, data-parallel over batch.

Strategy:
- Shard batch N=128 as 16 rows per core; zero collectives during the scan.
- Per-core state layout is h-major folded: [128 partitions, 4*16] where
  h = k*128 + p lives at (p, k*16 + n). Elementwise ops run on [128,64]
  tiles; matmul rhs k-chunks are contiguous [128,16] slices of the same
  tile, and matmul outputs land back in the identical layout.
- Layer-1 drive (W1*x + b1) and the three rhythm masks are precomputed on
  host in this layout and streamed from HBM in double-buffered blocks.
- Output layer is linear in s3, so osum = ss3 @ W4.T + T*b4 is computed on
  host from the accumulated spike counts; device only returns ss1/ss2/ss3.
"""
import numpy as np

T = 784
H = 512
NB = 16            # batch rows per core
NCORE = 8
B = 16             # timesteps per DMA block
NBLK = T // B
DECAY = 0.2
TH = 0.5
OUT = 10

_COMPILED = None


def _prep_wt(W):
    # [512out,512in] -> [128, 2048]: [p, k*512+c] = W.T[k*128+p, c]
    return np.ascontiguousarray(
        W.T.reshape(4, 128, 512).transpose(1, 0, 2).reshape(128, 2048),
        dtype=np.float32)


def _prep_c1(x_core, W1, b1):
    # [16,784] -> [NBLK, 128, B*64] with value W1[h]*x[n,t]+b1[h] at
    # (blk, p, ti*64 + k*16 + n), h = k*128+p, t = blk*B+ti
    C = W1[:, 0][:, None, None] * x_core[None, :, :] + b1[:, None, None]
    C = C.reshape(4, 128, NB, NBLK, B).transpose(3, 1, 4, 0, 2)
    return np.ascontiguousarray(C.reshape(NBLK, 128, B * 64), dtype=np.float32)


def _prep_mask(m):
    M = m.reshape(4, 128, NBLK, B).transpose(2, 1, 3, 0)
    M = np.broadcast_to(M[..., None], (NBLK, 128, B, 4, NB))
    return np.ascontiguousarray(M.reshape(NBLK, 128, B * 64), dtype=np.float32)


def _from_hmaj(a, n):
    # [128, 4n] -> [512, n]
    return a.reshape(128, 4, n).transpose(1, 0, 2).reshape(512, n)


def _build():
    import concourse.tile as tile
    from concourse import bacc, mybir
    from contextlib import ExitStack

    f32 = mybir.dt.float32
    Alu = mybir.AluOpType

    nc = bacc.Bacc("TRN2", target_bir_lowering=False, debug=False,
                   num_devices=NCORE)

    c1_d = nc.dram_tensor("c1s", [NBLK, 128, B * 64], f32, kind="ExternalInput").ap()
    m_d = [nc.dram_tensor(f"m{l}s", [NBLK, 128, B * 64], f32, kind="ExternalInput").ap()
           for l in (1, 2, 3)]
    wt2_d = nc.dram_tensor("wt2", [128, 2048], f32, kind="ExternalInput").ap()
    wt3_d = nc.dram_tensor("wt3", [128, 2048], f32, kind="ExternalInput").ap()
    b2_d = nc.dram_tensor("b2c", [128, 4], f32, kind="ExternalInput").ap()
    b3_d = nc.dram_tensor("b3c", [128, 4], f32, kind="ExternalInput").ap()
    ss_d = [nc.dram_tensor(f"ss{l}o", [128, 64], f32, kind="ExternalOutput").ap()
            for l in (1, 2, 3)]

    with ExitStack() as ctx:
        tc = ctx.enter_context(tile.TileContext(nc))
        consts = ctx.enter_context(tc.tile_pool(name="consts", bufs=1))
        states = ctx.enter_context(tc.tile_pool(name="states", bufs=1))
        spool = ctx.enter_context(tc.tile_pool(name="spool", bufs=2))
        stream = ctx.enter_context(tc.tile_pool(name="stream", bufs=2))
        tmp = ctx.enter_context(tc.tile_pool(name="tmp", bufs=3))
        psum = ctx.enter_context(tc.tile_pool(name="psum", bufs=2, space="PSUM"))

        wt2 = consts.tile([128, 2048], f32, tag="wt2")
        nc.default_dma_engine.dma_start(out=wt2[:], in_=wt2_d)
        wt3 = consts.tile([128, 2048], f32, tag="wt3")
        nc.default_dma_engine.dma_start(out=wt3[:], in_=wt3_d)
        b2s = consts.tile([128, 4], f32, tag="b2s")
        nc.default_dma_engine.dma_start(out=b2s[:], in_=b2_d)
        b3s = consts.tile([128, 4], f32, tag="b3s")
        nc.default_dma_engine.dma_start(out=b3s[:], in_=b3_d)

        u = [states.tile([128, 64], f32, tag=f"u{l}", name=f"u{l}") for l in range(3)]
        ss = [states.tile([128, 64], f32, tag=f"ss{l}", name=f"ss{l}") for l in range(3)]
        s = [spool.tile([128, 64], f32, tag=f"s{l}", name=f"s{l}") for l in range(3)]
        for t_ in u + ss + s:
            nc.vector.memset(t_[:], 0.0)

        wts = [None, wt2, wt3]
        bs = [None, b2s, b3s]

        def layer_update(li, zz, Mt):
            # u' = u + M*(D*u*(1-s) + zz - u);  s' = (u' > TH)*M;  ss += s'
            nw = tmp.tile([128, 64], f32, tag="nw", name="nw")
            nc.vector.scalar_tensor_tensor(nw[:], s[li][:], 1.0, u[li][:],
                                           Alu.subtract, Alu.mult)
            q = tmp.tile([128, 64], f32, tag="q", name="q")
            nc.vector.scalar_tensor_tensor(q[:], nw[:], -DECAY, u[li][:],
                                           Alu.mult, Alu.subtract)
            d = tmp.tile([128, 64], f32, tag="d", name="d")
            nc.vector.tensor_tensor(d[:], q[:], zz[:], Alu.add)
            md = tmp.tile([128, 64], f32, tag="md", name="md")
            nc.gpsimd.tensor_tensor(md[:], d[:], Mt[:], Alu.mult)
            nc.vector.tensor_tensor(u[li][:], u[li][:], md[:], Alu.add)
            snew = spool.tile([128, 64], f32, tag=f"s{li}", name=f"s{li}")
            nc.vector.scalar_tensor_tensor(snew[:], u[li][:], TH, Mt[:],
                                           Alu.is_gt, Alu.mult)
            s[li] = snew
            nc.gpsimd.tensor_tensor(ss[li][:], ss[li][:], snew[:], Alu.add)

        def matmul_layer(li, zz_tag):
            z = psum.tile([128, 64], f32, tag=f"z{li}", name=f"z{li}")
            first = True
            for j in range(4):
                for k in range(4):
                    nc.tensor.matmul(
                        z[:, j * 16:(j + 1) * 16],
                        wts[li][:, k * 512 + j * 128: k * 512 + j * 128 + 128],
                        s[li - 1][:, k * 16:(k + 1) * 16],
                        start=first, stop=(j == 3 and k == 3),
                        skip_group_check=True)
                    first = False
            zz = tmp.tile([128, 64], f32, tag=zz_tag, name=zz_tag)
            for j in range(4):
                nc.scalar.add(zz[:, j * 16:(j + 1) * 16],
                              z[:, j * 16:(j + 1) * 16],
                              bs[li][:, j:j + 1])
            return zz

        for blk in range(NBLK):
            c1b = stream.tile([128, B * 64], f32, tag="c1b")
            nc.default_dma_engine.dma_start(out=c1b[:], in_=c1_d[blk])
            mb = []
            for l in range(3):
                t_ = stream.tile([128, B * 64], f32, tag=f"m{l}b", name=f"m{l}b")
                nc.default_dma_engine.dma_start(out=t_[:], in_=m_d[l][blk])
                mb.append(t_)
            for ti in range(B):
                sl = slice(ti * 64, ti * 64 + 64)
                layer_update(0, c1b[:, sl], mb[0][:, sl])
                zz2 = matmul_layer(1, "zz2")
                layer_update(1, zz2, mb[1][:, sl])
                zz3 = matmul_layer(2, "zz3")
                layer_update(2, zz3, mb[2][:, sl])

        for l in range(3):
            nc.default_dma_engine.dma_start(out=ss_d[l], in_=ss[l][:])
    nc.compile()
    return nc


def _get_compiled():
    global _COMPILED
    if _COMPILED is None:
        _COMPILED = _build()
    return _COMPILED


def kernel(x, W1, b1, W2, b2, W3, b3, W4, b4, mask1, mask2, mask3):
    from concourse.bass_utils import run_bass_kernel_spmd

    x = np.asarray(x, np.float32)
    nc = _get_compiled()

    wt2 = _prep_wt(np.asarray(W2, np.float32))
    wt3 = _prep_wt(np.asarray(W3, np.float32))
    b2c = np.ascontiguousarray(np.asarray(b2, np.float32).reshape(4, 128).T)
    b3c = np.ascontiguousarray(np.asarray(b3, np.float32).reshape(4, 128).T)
    ms = [_prep_mask(np.asarray(m, np.float32)) for m in (mask1, mask2, mask3)]

    in_maps = []
    for c in range(NCORE):
        xc = x[c * NB:(c + 1) * NB]
        in_maps.append({
            "c1s": _prep_c1(xc, np.asarray(W1, np.float32),
                            np.asarray(b1, np.float32)),
            "m1s": ms[0], "m2s": ms[1], "m3s": ms[2],
            "wt2": wt2, "wt3": wt3, "b2c": b2c, "b3c": b3c,
        })

    res = run_bass_kernel_spmd(nc, in_maps, core_ids=list(range(NCORE)))
    outs = res.results

    frs = []
    for l in range(3):
        parts = [_from_hmaj(outs[c][f"ss{l + 1}o"], NB) for c in range(NCORE)]
        full = np.concatenate(parts, axis=1).T  # [128, 512]
        frs.append((full / np.float32(T)).astype(np.float32))
    outputs = (frs[2] @ np.asarray(W4, np.float32).T
               + np.asarray(b4, np.float32)).astype(np.float32)
    layer_fr = np.stack([np.float32(f.sum() / (128 * H)) for f in frs])
    return outputs, frs[0], frs[1], frs[2], layer_fr
